# revision 6
# baseline (speedup 1.0000x reference)
"""Dual-branch attention (shared attn weights, se/de value branches) on 8 TRN2 cores.

Sharding: 2 batches x 16 heads = 32 (b,h) pairs; core i owns batch i//4 and
heads [4*(i%4), 4*(i%4)+4) (128 feature channels). Activations are passed
pre-transposed ([C, N]) and in bf16 so the per-core kernel needs no on-chip
transposes. Each core computes its heads' attention for both value branches
and a row-sharded partial of the output projections; the host sums the 4
partials per batch and adds the biases.
"""

from contextlib import ExitStack

import numpy as np
import ml_dtypes

import concourse.bass as bass
import concourse.mybir as mybir
import concourse.tile as tile
from concourse import bacc
from concourse.bass import ts, ds
from concourse.bass_utils import run_bass_kernel_spmd

B, N, C, H, D = 2, 2048, 512, 16, 32
SCALE = D ** -0.5
P = 128
CJ = C // P      # 4 contraction chunks for the projections
NJ = 4           # q blocks of 512
KJ = N // NJ     # 512
NK = N // P      # 16 k chunks of 128
HL = 4           # heads per core
F = HL * D       # 128 local feature channels
VW = 2 * D + 1   # per-head vpack width: [v_se | v_de | ones]

BF16 = mybir.dt.bfloat16
F32 = mybir.dt.float32
NPBF16 = ml_dtypes.bfloat16


def build_nc(use_f32r=False):
    nc = bacc.Bacc("TRN2", target_bir_lowering=False, debug=False, num_devices=8)

    sT = nc.dram_tensor("sT", [C, N], BF16, kind="ExternalInput").ap()
    dT = nc.dram_tensor("dT", [C, N], BF16, kind="ExternalInput").ap()
    wq = nc.dram_tensor("wq", [C, F], BF16, kind="ExternalInput").ap()
    wk = nc.dram_tensor("wk", [C, F], BF16, kind="ExternalInput").ap()
    wvs = nc.dram_tensor("wvs", [C, F], BF16, kind="ExternalInput").ap()
    wvd = nc.dram_tensor("wvd", [C, F], BF16, kind="ExternalInput").ap()
    wps = nc.dram_tensor("wps", [F, C], BF16, kind="ExternalInput").ap()
    wpd = nc.dram_tensor("wpd", [F, C], BF16, kind="ExternalInput").ap()
    out = nc.dram_tensor("out", [2, N, C], F32, kind="ExternalOutput").ap()

    EXP = mybir.ActivationFunctionType.Exp
    MUL = mybir.AluOpType.mult

    with ExitStack() as ctx:
        tc = ctx.enter_context(tile.TileContext(nc))
        consts = ctx.enter_context(tc.tile_pool(name="consts", bufs=1))
        ppool = ctx.enter_context(tc.tile_pool(name="probs", bufs=40))
        stg = ctx.enter_context(tc.tile_pool(name="stg", bufs=2))
        ps_s = ctx.enter_context(tc.tile_pool(name="ps_s", bufs=3, space="PSUM"))
        ps_o = ctx.enter_context(tc.tile_pool(name="ps_o", bufs=2, space="PSUM"))
        ps_r = ctx.enter_context(tc.tile_pool(name="ps_r", bufs=1, space="PSUM"))
        ps_p = ctx.enter_context(tc.tile_pool(name="ps_p", bufs=2, space="PSUM"))

        # ---- loads ----
        sTt = consts.tile([P, CJ, N], BF16)
        dTt = consts.tile([P, CJ, N], BF16)
        sT3 = sT.rearrange("(co p) n -> p co n", p=P)
        dT3 = dT.rearrange("(co p) n -> p co n", p=P)
        for c in range(CJ):
            nc.sync.dma_start(sTt[:, c], sT3[:, c])
            nc.sync.dma_start(dTt[:, c], dT3[:, c])
        wqt = consts.tile([P, CJ, F], BF16, tag="wq")
        wkt = consts.tile([P, CJ, F], BF16, tag="wk")
        wvst = consts.tile([P, CJ, F], BF16, tag="wvs")
        wvdt = consts.tile([P, CJ, F], BF16, tag="wvd")
        for w_ap, w_t in ((wq, wqt), (wk, wkt), (wvs, wvst), (wvd, wvdt)):
            nc.sync.dma_start(w_t[:], w_ap.rearrange("(co p) f -> p co f", p=P))
        wpst = consts.tile([P, C], BF16, tag="wps")
        wpdt = consts.tile([P, C], BF16, tag="wpd")
        nc.sync.dma_start(wpst[:], wps)
        nc.sync.dma_start(wpdt[:], wpd)
        ones64 = consts.tile([1, 64], F32)
        nc.vector.memset(ones64[:], 1.0)

        def mm_dt(ap):
            return ap.bitcast(mybir.dt.float32r) if use_f32r else ap

        # ---- q/k projections into transposed [feat, N] layout ----
        # Matmul APs may only start at partition 0/32/64, so the 4 local heads
        # are stored as two head-pair tiles of 64 partitions (bases 0 and 32).
        qtA = consts.tile([64, N], BF16, tag="qtA")
        qtB = consts.tile([64, N], BF16, tag="qtB")
        ktA = consts.tile([64, N], BF16, tag="ktA")
        ktB = consts.tile([64, N], BF16, tag="ktB")
        for w_t, dstA, dstB in ((wqt, qtA, qtB), (wkt, ktA, ktB)):
            for j in range(NJ):
                ps = ps_p.tile([P, KJ], F32, tag="pp")
                for c in range(CJ):
                    nc.tensor.matmul(
                        ps[:], w_t[:, c], sTt[:, c, ts(j, KJ)],
                        start=(c == 0), stop=(c == CJ - 1),
                    )
                nc.vector.tensor_copy(dstA[:, ts(j, KJ)], ps[0:64, :])
                nc.vector.tensor_copy(dstB[:, ts(j, KJ)], ps[64:P, :])

        # ---- value projections, natural [N, feat] layout, packed per head ----
        # vpk[:, n, h*VW:(h+1)*VW] = [v_se_h (32) | v_de_h (32) | ones (1)]
        vpk = consts.tile([P, NK, HL * VW], BF16)
        for n in range(NK):
            for br, (act, w_t) in enumerate(((sTt, wvst), (dTt, wvdt))):
                ps = ps_p.tile([P, KJ], F32, tag="pp")
                for c in range(CJ):
                    nc.tensor.matmul(
                        ps[:, :F], act[:, c, ts(n, P)], w_t[:, c],
                        start=(c == 0), stop=(c == CJ - 1),
                    )
                dst = vpk[:, n].rearrange("p (h y) -> p h y", h=HL)[:, :, br * D:(br + 1) * D]
                src = ps[:, :F].rearrange("p (h d) -> p h d", h=HL)
                nc.vector.tensor_copy(dst, src)
        nc.vector.memset(
            vpk.rearrange("p n (h y) -> p n h y", h=HL)[:, :, :, 2 * D:2 * D + 1], 1.0
        )

        # ---- attention ----
        outTs = consts.tile([P, N], BF16, tag="oTs")
        outTd = consts.tile([P, N], BF16, tag="oTd")

        def emit_qk_exp(j, h):
            qt = qtA if h < 2 else qtB
            kt = ktA if h < 2 else ktB
            hb = (h % 2) * D
            prs = []
            for m in range(NK):
                sp = ps_s.tile([P, KJ], F32, tag="sc")
                nc.tensor.matmul(
                    sp[:], kt[ds(hb, D), ts(m, P)], qt[ds(hb, D), ts(j, KJ)],
                    start=True, stop=True,
                )
                pr = ppool.tile([P, KJ], BF16, tag="pr")
                nc.scalar.activation(pr[:], sp[:], EXP, scale=SCALE)
                prs.append(pr)
            return prs

        def emit_pv_finish(j, h, prs):
            op = ps_o.tile([P, KJ], F32, tag="op")
            for m in range(NK):
                nc.tensor.matmul(
                    op[:VW, :], vpk[:, m, ds(h * VW, VW)], prs[m][:],
                    start=(m == 0), stop=(m == NK - 1),
                )
            rs = stg.tile([1, KJ], F32, tag="rs")
            nc.vector.reciprocal(rs[:], op[64:65, :])
            rb = ps_r.tile([64, KJ], F32, tag="rb")
            nc.tensor.matmul(rb[:], mm_dt(ones64[:]), mm_dt(rs[:]), start=True, stop=True)
            # DVE can read only one PSUM operand per op: stage rb in SBUF.
            rbs = stg.tile([64, KJ], F32, tag="rbs")
            nc.vector.tensor_copy(rbs[:], rb[:])
            nc.vector.tensor_tensor(
                outTs[ds(h * D, D), ts(j, KJ)], op[0:D, :], rbs[0:D, :], MUL)
            nc.vector.tensor_tensor(
                outTd[ds(h * D, D), ts(j, KJ)], op[D:2 * D, :], rbs[D:2 * D, :], MUL)

        def emit_outproj(j):
            out3 = [out[br].rearrange("(no p) c -> p no c", p=P) for br in range(2)]
            for br, (oT, wp_t) in enumerate(((outTs, wpst), (outTd, wpdt))):
                st = stg.tile([P, NJ, KJ], F32, tag="st")
                for nn in range(NJ):
                    pp = ps_p.tile([P, KJ], F32, tag="pp")
                    nc.tensor.matmul(
                        pp[:], oT[:, ds((NJ * j + nn) * P, P)], wp_t[:],
                        start=True, stop=True,
                    )
                    nc.vector.tensor_copy(st[:, nn], pp[:])
                nc.sync.dma_start(out3[br][:, ds(NJ * j, NJ)], st[:])

        prev = None
        for j in range(NJ):
            for h in range(HL):
                prs = emit_qk_exp(j, h)
                if prev is not None:
                    emit_pv_finish(*prev)
                    if prev[1] == HL - 1:
                        emit_outproj(prev[0])
                prev = (j, h, prs)
        emit_pv_finish(*prev)
        emit_outproj(prev[0])

    nc.compile()
    return nc


_NC_CACHE = {}


def _get_nc():
    if "nc" not in _NC_CACHE:
        _NC_CACHE["nc"] = build_nc()
    return _NC_CACHE["nc"]


def make_in_maps(se, de, W_qkv_se, W_v_de, W_proj_se, W_proj_de):
    se = np.asarray(se, dtype=np.float32)
    de = np.asarray(de, dtype=np.float32)
    W_qkv_se = np.asarray(W_qkv_se, dtype=np.float32)
    W_v_de = np.asarray(W_v_de, dtype=np.float32)
    W_proj_se = np.asarray(W_proj_se, dtype=np.float32)
    W_proj_de = np.asarray(W_proj_de, dtype=np.float32)
    qW, kW, vW = W_qkv_se[:, 0:C], W_qkv_se[:, C:2 * C], W_qkv_se[:, 2 * C:3 * C]
    sTs = [np.ascontiguousarray(se[b].T).astype(NPBF16) for b in range(B)]
    dTs = [np.ascontiguousarray(de[b].T).astype(NPBF16) for b in range(B)]
    in_maps = []
    for core in range(8):
        b, g = divmod(core, 4)
        sl = slice(g * F, (g + 1) * F)
        in_maps.append({
            "sT": sTs[b],
            "dT": dTs[b],
            "wq": np.ascontiguousarray(qW[:, sl]).astype(NPBF16),
            "wk": np.ascontiguousarray(kW[:, sl]).astype(NPBF16),
            "wvs": np.ascontiguousarray(vW[:, sl]).astype(NPBF16),
            "wvd": np.ascontiguousarray(W_v_de[:, sl]).astype(NPBF16),
            "wps": np.ascontiguousarray(W_proj_se[sl, :]).astype(NPBF16),
            "wpd": np.ascontiguousarray(W_proj_de[sl, :]).astype(NPBF16),
        })
    return in_maps


def gather_out(outs, b_proj_se, b_proj_de):
    b_proj_se = np.asarray(b_proj_se, dtype=np.float32)
    b_proj_de = np.asarray(b_proj_de, dtype=np.float32)
    out_se = np.stack(
        [sum(outs[4 * b + g][0] for g in range(4)) for b in range(B)]
    ) + b_proj_se[None, None, :]
    out_de = np.stack(
        [sum(outs[4 * b + g][1] for g in range(4)) for b in range(B)]
    ) + b_proj_de[None, None, :]
    return out_se.astype(np.float32), out_de.astype(np.float32)


def kernel(se, de, W_qkv_se, W_v_de, W_proj_se, b_proj_se, W_proj_de, b_proj_de):
    nc = _get_nc()
    in_maps = make_in_maps(se, de, W_qkv_se, W_v_de, W_proj_se, W_proj_de)
    res = run_bass_kernel_spmd(nc, in_maps, core_ids=list(range(8)))
    outs = [r["out"] for r in res.results]
    return gather_out(outs, b_proj_se, b_proj_de)
